# revision 6
# baseline (speedup 1.0000x reference)
"""Local (windowed) attention kernel for Trainium2, SPMD over 8 NeuronCores.

Problem (all shapes fixed):
  x [4, 4096, 1024] f32 -> qkv = x @ w_qkv; q,k,v = split(qkv)
  windows of 128 tokens attend to [prev window, own window] with a causal
  mask; NOTE the reference has a (faithful) bug: v2 = k2, so v is never
  used.  out = softmax(q k2^T / 32) @ k2 ; y = out @ w_out + b_out.

Sharding: data-parallel over (batch, seq-half): core c handles batch c//2,
tokens (c%2)*2048 ..+2048, with a 128-token halo (zeros at the front of a
batch, matching the reference's zero pad of k).

Key algebraic refactor (saves 1.48x matmul FLOPs vs projecting q,k):
  sim = q k^T / 32 = x (Wq Wk^T / 32) x^T = (x @ Wqk) x^T
  z   = k @ w_out  = x (Wk @ Wo)          =  x @ Wkv
with Wqk, Wkv precomputed on the host in f32.  The k projection vanishes;
x itself (SBUF-resident) serves as the sim moving operand and the z
stationary operand.

Device algorithm per core (all matmuls bf16 with fp32 PSUM accumulate):
  q'T = Wqk^T @ xT                  [1024, 2048]
  per 128-token tile t (17 incl. halo):
    z_t = xT_t^T @ Wkv              [128, 1024] token-major
  per 128-token window w (16), interleaved with the z tiles so PE never
  waits on the softmax chain:
    sim  = q'T_w^T xT_[w,w+2)      PSUM [128, 256]
    L    = sim + mask               (DVE, reads PSUM)
    E,s  = exp(L), rowsum           (ACT with accum_out, E in bf16)
    ET   = PE-transpose(E)          [2x 128x128]
    yps  = ET^T @ z_[w,w+2)        PSUM [128, 1024] (unnormalized)
    y    = yps * (1/s) + b_out      (one fused DVE op), DMA out (f32)
"""

import numpy as np
import ml_dtypes

B, N, DIN, DINNER, DOUT, W = 4, 4096, 1024, 1024, 1024, 128
NCORES = 8
TPC = 2048                # main (query) tokens per core
TKT = TPC + W             # tokens incl. halo = 2176
NWIN = TPC // W           # 16 windows per core
NT = TKT // 128           # 17 token tiles incl. halo
KD = DIN // 128           # 8 contraction tiles of 128
BF16 = ml_dtypes.bfloat16

# q' chunks in xT halo-inclusive columns (q tokens are cols 128..2176)
QCHUNKS = [(128, 512), (640, 512), (1152, 512), (1664, 512)]

_NC_CACHE = {}


def _build_nc():
    if "nc" in _NC_CACHE:
        return _NC_CACHE["nc"]

    import concourse.bacc as bacc
    import concourse.mybir as mybir
    import concourse.tile as tile
    from concourse.masks import make_identity

    f32 = mybir.dt.float32
    bf16 = mybir.dt.bfloat16

    nc = bacc.Bacc("TRN2", target_bir_lowering=False, debug=False)

    xT = nc.dram_tensor("xT", [DIN, TKT], bf16, kind="ExternalInput")
    wqk = nc.dram_tensor("wqk", [DIN, DINNER], bf16, kind="ExternalInput")
    wkv = nc.dram_tensor("wkv", [DIN, DOUT], bf16, kind="ExternalInput")
    bias = nc.dram_tensor("bias", [128, DOUT], bf16, kind="ExternalInput")
    mask = nc.dram_tensor("mask", [W, 2 * W], f32, kind="ExternalInput")
    y = nc.dram_tensor("y", [TPC, DOUT], f32, kind="ExternalOutput")

    from contextlib import ExitStack

    with tile.TileContext(nc) as tc, ExitStack() as ctx:
        consts = ctx.enter_context(tc.tile_pool(name="consts", bufs=1))
        resid = ctx.enter_context(tc.tile_pool(name="resid", bufs=1))
        wwin = ctx.enter_context(tc.tile_pool(name="wwin", bufs=4))
        ystage = ctx.enter_context(tc.tile_pool(name="ystage", bufs=3))
        pmm = ctx.enter_context(tc.tile_pool(name="pmm", bufs=4, space="PSUM"))
        psim = ctx.enter_context(tc.tile_pool(name="psim", bufs=2, space="PSUM"))
        ptr = ctx.enter_context(tc.tile_pool(name="ptr", bufs=2, space="PSUM"))

        # ---- tiles ----------------------------------------------------------
        wqk_sb = consts.tile([128, KD, DINNER], bf16)
        wkv_sb = consts.tile([128, KD, DOUT], bf16)
        bias_sb = consts.tile([128, DOUT], bf16)
        mask_sb = consts.tile([W, 2 * W], f32)
        ident = consts.tile([128, 128], bf16)

        xT_sb = resid.tile([128, KD, TKT], bf16)
        qT_sb = resid.tile([128, KD, TPC], bf16)
        z_sb = resid.tile([128, NT, DOUT], bf16)

        # PE is data-starved for the first ~8us (DMA init + first chunk
        # arrival) and HAM holds it at half clock for its first ~3.4us of
        # sustained work.  Burn the idle window on dummy matmuls over a
        # memset tile so the clock gate opens before real data lands.
        warm = consts.tile([128, 512], bf16)
        nc.vector.memset(warm[:], 0.0)
        wps = pmm.tile([128, 512], f32, tag="mm")
        for i in range(8):
            nc.tensor.matmul(
                wps[:], warm[:, 0:128], warm[:], start=(i == 0), stop=(i == 7)
            )

        # ---- DMAs -----------------------------------------------------------
        # Few, large issues (each dma_start costs ~0.8us on the sync
        # sequencer).  The m-th q' matmul group gates on (wqk m-slice m) +
        # (xT chunk-0 cols): the m-slices land ~0.7us apart while a group
        # takes ~1.7us of PE, so DMA stays ahead after the first group.
        wqk_r = wqk.rearrange("(k p) n -> p k n", p=128)
        wkv_r = wkv.rearrange("(k p) n -> p k n", p=128)
        xT_r = xT.rearrange("(k p) n -> p k n", p=128)
        nc.sync.dma_start(wqk_sb[:, :, 0:128], wqk_r[:, :, 0:128])
        nc.sync.dma_start(xT_sb[:, :, 128:640], xT_r[:, :, 128:640])
        for m in range(1, KD):
            nc.sync.dma_start(
                wqk_sb[:, :, 128 * m : 128 * (m + 1)],
                wqk_r[:, :, 128 * m : 128 * (m + 1)],
            )

        # ---- phase 1: q'T projection ---------------------------------------
        for ci, (c0, cn) in enumerate(QCHUNKS):
            for m in range(KD):
                ps = pmm.tile([128, 512], f32, tag="mm")
                for k in range(KD):
                    nc.tensor.matmul(
                        ps[:, :cn],
                        wqk_sb[:, k, 128 * m : 128 * (m + 1)],
                        xT_sb[:, k, c0 : c0 + cn],
                        start=(k == 0),
                        stop=(k == KD - 1),
                    )
                nc.vector.tensor_copy(qT_sb[:, m, c0 - W : c0 - W + cn], ps[:, :cn])
            if ci == 0:
                # issued after chunk-0 matmuls: overlap with that compute
                nc.sync.dma_start(xT_sb[:, :, 640:1152], xT_r[:, :, 640:1152])
                nc.sync.dma_start(xT_sb[:, :, 1152:], xT_r[:, :, 1152:])
                nc.sync.dma_start(xT_sb[:, :, 0:128], xT_r[:, :, 0:128])
            elif ci == 1:
                nc.sync.dma_start(wkv_sb[:], wkv_r[:])
                nc.sync.dma_start(bias_sb[:], bias[:])
                nc.sync.dma_start(mask_sb[:], mask[:])
                make_identity(nc, ident)

        # ---- phase 2: z tiles interleaved with attention windows -----------
        def z_half(t, nh):
            ps = pmm.tile([128, 512], f32, tag="mm")
            for k in range(KD):
                nc.tensor.matmul(
                    ps[:],
                    xT_sb[:, k, 128 * t : 128 * (t + 1)],
                    wkv_sb[:, k, 512 * nh : 512 * (nh + 1)],
                    start=(k == 0),
                    stop=(k == KD - 1),
                )
            nc.scalar.copy(z_sb[:, t, 512 * nh : 512 * (nh + 1)], ps[:])

        def z_tile(t):
            z_half(t, 0)
            z_half(t, 1)

        z_tile(0)
        z_tile(1)
        for w in range(NWIN):
            sim = psim.tile([128, 2 * W], f32, tag="sim")
            for k in range(KD):
                nc.tensor.matmul(
                    sim[:],
                    qT_sb[:, k, W * w : W * (w + 1)],
                    xT_sb[:, k, W * w : W * (w + 2)],
                    start=(k == 0),
                    stop=(k == KD - 1),
                )
            L = wwin.tile([128, 2 * W], f32, tag="L")
            nc.vector.tensor_tensor(L[:], sim[:], mask_sb[:], op=_alu().add)
            E = wwin.tile([128, 2 * W], bf16, tag="E")
            s = wwin.tile([128, 1], f32, tag="s")
            nc.scalar.activation(E[:], L[:], _act().Exp, accum_out=s[:])
            r = wwin.tile([128, 1], f32, tag="r")
            nc.vector.reciprocal(r[:], s[:])
            # the next z tile here keeps PE busy while DVE/ACT produce E;
            # the last z tile is split across the final two windows so the
            # exp latency stays hidden through w=15
            if w + 2 < NT - 1:
                z_tile(w + 2)
            elif w == NWIN - 2:
                z_half(NT - 1, 0)
            elif w == NWIN - 1:
                z_half(NT - 1, 1)
            # transpose E -> ET [j, i] (two 128x128 blocks)
            et_ps = ptr.tile([128, 2, 128], bf16, tag="tr")
            nc.tensor.transpose(et_ps[:, 0, :], E[:, 0:128], ident[:])
            nc.tensor.transpose(et_ps[:, 1, :], E[:, 128:256], ident[:])
            ET = wwin.tile([128, 2, 128], bf16, tag="ET")
            nc.vector.tensor_copy(ET[:], et_ps[:])
            # y_w = ET^T @ z[w:w+2], normalized + bias on eviction
            yt = ystage.tile([128, DOUT], f32, tag="y")
            for nh in range(2):
                ps = pmm.tile([128, 512], f32, tag="mm")
                for jt in range(2):
                    nc.tensor.matmul(
                        ps[:],
                        ET[:, jt, :],
                        z_sb[:, w + jt, 512 * nh : 512 * (nh + 1)],
                        start=(jt == 0),
                        stop=(jt == 1),
                    )
                nc.vector.scalar_tensor_tensor(
                    yt[:, 512 * nh : 512 * (nh + 1)],
                    ps[:],
                    r[:],
                    bias_sb[:, 512 * nh : 512 * (nh + 1)],
                    op0=_alu().mult,
                    op1=_alu().add,
                )
            nc.sync.dma_start(y[W * w : W * (w + 1), :], yt[:])

    nc.compile()
    _NC_CACHE["nc"] = nc
    return nc


def _alu():
    import concourse.mybir as mybir

    return mybir.AluOpType


def _act():
    import concourse.mybir as mybir

    return mybir.ActivationFunctionType


def _make_mask():
    # row i (query), col j of [prev, cur]: masked (set very negative)
    # where j > i + W  (strictly causal within the 2-window lookback)
    i = np.arange(W)[:, None]
    j = np.arange(2 * W)[None, :]
    return np.where(j > i + W, np.float32(-1e30), np.float32(0.0))


def prep_in_maps(x, w_qkv, w_out, b_out):
    scale = np.float32(DINNER) ** np.float32(-0.5)
    wq = np.asarray(w_qkv[:, :DINNER], dtype=np.float32)
    wk = np.asarray(w_qkv[:, DINNER : 2 * DINNER], dtype=np.float32)
    wo = np.asarray(w_out, dtype=np.float32)
    wqk = ((wq @ wk.T) * scale).astype(BF16)
    wkv = (wk @ wo).astype(BF16)
    bias = np.broadcast_to(b_out.astype(BF16), (128, DOUT)).copy()
    mask = _make_mask()
    in_maps = []
    for c in range(NCORES):
        b, h = divmod(c, 2)
        xTc = np.zeros((DIN, TKT), dtype=BF16)
        xb = np.ascontiguousarray(x[b].T)  # [DIN, N]
        xTc[:, W:] = xb[:, h * TPC : (h + 1) * TPC].astype(BF16)
        if h == 1:
            xTc[:, :W] = xb[:, TPC - W : TPC].astype(BF16)
        in_maps.append(
            {"xT": xTc, "wqk": wqk, "wkv": wkv, "bias": bias, "mask": mask}
        )
    return in_maps


def kernel(x, w_qkv, w_out, b_out, _trace=False):
    from concourse import bass_utils

    x = np.asarray(x)
    w_qkv = np.asarray(w_qkv)
    w_out = np.asarray(w_out)
    b_out = np.asarray(b_out)

    nc = _build_nc()
    in_maps = prep_in_maps(x, w_qkv, w_out, b_out)
    res = bass_utils.run_bass_kernel_spmd(
        nc, in_maps, core_ids=list(range(NCORES)), trace=_trace
    )
    out = np.empty((B, N, DOUT), dtype=np.float32)
    for c in range(NCORES):
        b, h = divmod(c, 2)
        out[b, h * TPC : (h + 1) * TPC, :] = res.results[c]["y"]
    if _trace:
        kernel.last_exec_time_ns = res.exec_time_ns
        kernel.last_results = res
    return out


# revision 8
# speedup vs baseline: 1.0209x; 1.0209x over previous
"""Local (windowed) attention kernel for Trainium2, SPMD over 8 NeuronCores.

Problem (all shapes fixed):
  x [4, 4096, 1024] f32 -> qkv = x @ w_qkv; q,k,v = split(qkv)
  windows of 128 tokens attend to [prev window, own window] with a causal
  mask; NOTE the reference has a (faithful) bug: v2 = k2, so v is never
  used.  out = softmax(q k2^T / 32) @ k2 ; y = out @ w_out + b_out.

Sharding: data-parallel over (batch, seq-half): core c handles batch c//2,
tokens (c%2)*2048 ..+2048, with a 128-token halo (zeros at the front of a
batch, matching the reference's zero pad of k).

Key algebraic refactor (saves 1.48x matmul FLOPs vs projecting q,k):
  sim = q k^T / 32 = x (Wq Wk^T / 32) x^T = (x @ Wqk) x^T
  z   = k @ w_out  = x (Wk @ Wo)          =  x @ Wkv
with Wqk, Wkv precomputed on the host in f32.  The k projection vanishes;
x itself (SBUF-resident) serves as the sim moving operand and the z
stationary operand.

Device algorithm per core (all matmuls bf16 with fp32 PSUM accumulate):
  q'T = Wqk^T @ xT                  [1024, 2048]
  per 128-token tile t (17 incl. halo):
    z_t = xT_t^T @ Wkv              [128, 1024] token-major
  per 128-token window w (16), interleaved with the z tiles so PE never
  waits on the softmax chain:
    sim  = q'T_w^T xT_[w,w+2)      PSUM [128, 256]
    L    = sim + mask               (DVE, reads PSUM)
    E,s  = exp(L), rowsum           (ACT with accum_out, E in bf16)
    ET   = PE-transpose(E)          [2x 128x128]
    yps  = ET^T @ z_[w,w+2)        PSUM [128, 1024] (unnormalized)
    y    = yps * (1/s) + b_out      (one fused DVE op), DMA out (f32)
"""

import numpy as np
import ml_dtypes

B, N, DIN, DINNER, DOUT, W = 4, 4096, 1024, 1024, 1024, 128
NCORES = 8
TPC = 2048                # main (query) tokens per core
TKT = TPC + W             # tokens incl. halo = 2176
NWIN = TPC // W           # 16 windows per core
NT = TKT // 128           # 17 token tiles incl. halo
KD = DIN // 128           # 8 contraction tiles of 128
BF16 = ml_dtypes.bfloat16

# q' chunks in xT halo-inclusive columns (q tokens are cols 128..2176)
QCHUNKS = [(128, 512), (640, 512), (1152, 512), (1664, 512)]

_NC_CACHE = {}


def _build_nc():
    if "nc" in _NC_CACHE:
        return _NC_CACHE["nc"]

    import concourse.bacc as bacc
    import concourse.mybir as mybir
    import concourse.tile as tile
    from concourse.masks import make_identity

    f32 = mybir.dt.float32
    bf16 = mybir.dt.bfloat16

    nc = bacc.Bacc("TRN2", target_bir_lowering=False, debug=False)

    xT = nc.dram_tensor("xT", [DIN, TKT], bf16, kind="ExternalInput")
    wqk = nc.dram_tensor("wqk", [DIN, DINNER], bf16, kind="ExternalInput")
    wkv = nc.dram_tensor("wkv", [DIN, DOUT], bf16, kind="ExternalInput")
    bias = nc.dram_tensor("bias", [128, DOUT], bf16, kind="ExternalInput")
    mask = nc.dram_tensor("mask", [W, 2 * W], f32, kind="ExternalInput")
    y = nc.dram_tensor("y", [TPC, DOUT], f32, kind="ExternalOutput")

    from contextlib import ExitStack

    with tile.TileContext(nc) as tc, ExitStack() as ctx:
        consts = ctx.enter_context(tc.tile_pool(name="consts", bufs=1))
        resid = ctx.enter_context(tc.tile_pool(name="resid", bufs=1))
        wwin = ctx.enter_context(tc.tile_pool(name="wwin", bufs=4))
        ystage = ctx.enter_context(tc.tile_pool(name="ystage", bufs=3))
        pmm = ctx.enter_context(tc.tile_pool(name="pmm", bufs=4, space="PSUM"))
        psim = ctx.enter_context(tc.tile_pool(name="psim", bufs=2, space="PSUM"))
        ptr = ctx.enter_context(tc.tile_pool(name="ptr", bufs=2, space="PSUM"))

        # ---- tiles ----------------------------------------------------------
        wqk_sb = consts.tile([128, KD, DINNER], bf16)
        wkv_sb = consts.tile([128, KD, DOUT], bf16)
        bias_sb = consts.tile([128, DOUT], bf16)
        mask_sb = consts.tile([W, 2 * W], f32)
        ident = consts.tile([128, 128], bf16)

        xT_sb = resid.tile([128, KD, TKT], bf16)
        qT_sb = resid.tile([128, KD, TPC], bf16)
        z_sb = resid.tile([128, NT, DOUT], bf16)

        # PE is data-starved for the first ~8us (DMA init + first chunk
        # arrival) and HAM holds it at half clock for its first ~3.4us of
        # sustained work.  Burn the idle window on dummy matmuls over a
        # memset tile so the clock gate opens before real data lands.
        warm = consts.tile([128, 512], bf16)
        nc.gpsimd.memset(warm[:], 0.0)
        wps = pmm.tile([128, 512], f32, tag="mm")
        for i in range(6):
            nc.tensor.matmul(
                wps[:], warm[:, 0:128], warm[:], start=(i == 0), stop=(i == 5)
            )

        # ---- DMAs -----------------------------------------------------------
        # Few, large issues (each dma_start costs ~0.8us on the sync
        # sequencer; lines must stay >= 1KB or descriptor-processing
        # dominates).  Ordered so the first z halves gate on ~1.3MB only,
        # then each successive z half / q' group chases its arrival.
        wqk_r = wqk.rearrange("(k p) n -> p k n", p=128)
        wkv_r = wkv.rearrange("(k p) n -> p k n", p=128)
        xT_r = xT.rearrange("(k p) n -> p k n", p=128)
        nc.sync.dma_start(wkv_sb[:, :, 0:512], wkv_r[:, :, 0:512])
        nc.sync.dma_start(xT_sb[:, :, 0:128], xT_r[:, :, 0:128])
        nc.sync.dma_start(wkv_sb[:, :, 512:], wkv_r[:, :, 512:])
        nc.sync.dma_start(xT_sb[:, :, 128:384], xT_r[:, :, 128:384])
        nc.sync.dma_start(xT_sb[:, :, 384:640], xT_r[:, :, 384:640])
        nc.sync.dma_start(wqk_sb[:], wqk_r[:])
        nc.sync.dma_start(xT_sb[:, :, 640:1152], xT_r[:, :, 640:1152])
        nc.sync.dma_start(bias_sb[:], bias[:])
        nc.sync.dma_start(mask_sb[:], mask[:])
        nc.sync.dma_start(xT_sb[:, :, 1152:], xT_r[:, :, 1152:])

        # ---- compute building blocks ----------------------------------------
        def qc_group(c0, cn, m):
            ps = pmm.tile([128, 512], f32, tag="mm")
            for k in range(KD):
                nc.tensor.matmul(
                    ps[:, :cn],
                    wqk_sb[:, k, 128 * m : 128 * (m + 1)],
                    xT_sb[:, k, c0 : c0 + cn],
                    start=(k == 0),
                    stop=(k == KD - 1),
                )
            nc.vector.tensor_copy(qT_sb[:, m, c0 - W : c0 - W + cn], ps[:, :cn])

        def z_half(t, nh):
            ps = pmm.tile([128, 512], f32, tag="mm")
            for k in range(KD):
                nc.tensor.matmul(
                    ps[:],
                    xT_sb[:, k, 128 * t : 128 * (t + 1)],
                    wkv_sb[:, k, 512 * nh : 512 * (nh + 1)],
                    start=(k == 0),
                    stop=(k == KD - 1),
                )
            nc.scalar.copy(z_sb[:, t, 512 * nh : 512 * (nh + 1)], ps[:])

        def z_tile(t):
            z_half(t, 0)
            z_half(t, 1)

        # ---- schedule --------------------------------------------------------
        # z tiles 0..5 first (smallest DMA gate), then q' chunk 0 (wqk has
        # landed by then).  Each window slot then carries ~2-4us of fill
        # (two q' m-groups and/or the z tile for window w+1) between its sim
        # and its transpose, hiding the mask/exp latency on DVE/ACT.
        for t in range(6):
            z_tile(t)
        for m in range(KD):
            qc_group(128, 512, m)
        make_identity(nc, ident)

        for w in range(NWIN):
            sim = psim.tile([128, 2 * W], f32, tag="sim")
            for k in range(KD):
                nc.tensor.matmul(
                    sim[:],
                    qT_sb[:, k, W * w : W * (w + 1)],
                    xT_sb[:, k, W * w : W * (w + 2)],
                    start=(k == 0),
                    stop=(k == KD - 1),
                )
            L = wwin.tile([128, 2 * W], f32, tag="L")
            nc.vector.tensor_tensor(L[:], sim[:], mask_sb[:], op=_alu().add)
            E = wwin.tile([128, 2 * W], bf16, tag="E")
            s = wwin.tile([128, 1], f32, tag="s")
            nc.scalar.activation(E[:], L[:], _act().Exp, accum_out=s[:])
            r = wwin.tile([128, 1], f32, tag="r")
            nc.vector.reciprocal(r[:], s[:])
            # fills: z tile w+1 just-in-time for this window's attn; q'
            # chunks 1..3 spread two m-groups per slot over w0..w11
            if w >= 5:
                z_tile(w + 1)
            if w < 12:
                c0 = 640 + 512 * (w // 4)
                m0 = 2 * (w % 4)
                qc_group(c0, 512, m0)
                qc_group(c0, 512, m0 + 1)
            # transpose E -> ET [j, i] (two 128x128 blocks)
            et_ps = ptr.tile([128, 2, 128], bf16, tag="tr")
            nc.tensor.transpose(et_ps[:, 0, :], E[:, 0:128], ident[:])
            nc.tensor.transpose(et_ps[:, 1, :], E[:, 128:256], ident[:])
            ET = wwin.tile([128, 2, 128], bf16, tag="ET")
            nc.vector.tensor_copy(ET[:], et_ps[:])
            # y_w = ET^T @ z[w:w+2], normalized + bias on eviction; each
            # 512-half DMAs out as soon as its fused scale+bias lands
            yt = ystage.tile([128, DOUT], f32, tag="y")
            for nh in range(2):
                ps = pmm.tile([128, 512], f32, tag="mm")
                for jt in range(2):
                    nc.tensor.matmul(
                        ps[:],
                        ET[:, jt, :],
                        z_sb[:, w + jt, 512 * nh : 512 * (nh + 1)],
                        start=(jt == 0),
                        stop=(jt == 1),
                    )
                nc.vector.scalar_tensor_tensor(
                    yt[:, 512 * nh : 512 * (nh + 1)],
                    ps[:],
                    r[:],
                    bias_sb[:, 512 * nh : 512 * (nh + 1)],
                    op0=_alu().mult,
                    op1=_alu().add,
                )
                nc.sync.dma_start(
                    y[W * w : W * (w + 1), 512 * nh : 512 * (nh + 1)],
                    yt[:, 512 * nh : 512 * (nh + 1)],
                )

    nc.compile()
    _NC_CACHE["nc"] = nc
    return nc


def _alu():
    import concourse.mybir as mybir

    return mybir.AluOpType


def _act():
    import concourse.mybir as mybir

    return mybir.ActivationFunctionType


def _make_mask():
    # row i (query), col j of [prev, cur]: masked (set very negative)
    # where j > i + W  (strictly causal within the 2-window lookback)
    i = np.arange(W)[:, None]
    j = np.arange(2 * W)[None, :]
    return np.where(j > i + W, np.float32(-1e30), np.float32(0.0))


def prep_in_maps(x, w_qkv, w_out, b_out):
    scale = np.float32(DINNER) ** np.float32(-0.5)
    wq = np.asarray(w_qkv[:, :DINNER], dtype=np.float32)
    wk = np.asarray(w_qkv[:, DINNER : 2 * DINNER], dtype=np.float32)
    wo = np.asarray(w_out, dtype=np.float32)
    wqk = ((wq @ wk.T) * scale).astype(BF16)
    wkv = (wk @ wo).astype(BF16)
    bias = np.broadcast_to(b_out.astype(BF16), (128, DOUT)).copy()
    mask = _make_mask()
    in_maps = []
    for c in range(NCORES):
        b, h = divmod(c, 2)
        xTc = np.zeros((DIN, TKT), dtype=BF16)
        xb = np.ascontiguousarray(x[b].T)  # [DIN, N]
        xTc[:, W:] = xb[:, h * TPC : (h + 1) * TPC].astype(BF16)
        if h == 1:
            xTc[:, :W] = xb[:, TPC - W : TPC].astype(BF16)
        in_maps.append(
            {"xT": xTc, "wqk": wqk, "wkv": wkv, "bias": bias, "mask": mask}
        )
    return in_maps


def kernel(x, w_qkv, w_out, b_out, _trace=False):
    from concourse import bass_utils

    x = np.asarray(x)
    w_qkv = np.asarray(w_qkv)
    w_out = np.asarray(w_out)
    b_out = np.asarray(b_out)

    nc = _build_nc()
    in_maps = prep_in_maps(x, w_qkv, w_out, b_out)
    res = bass_utils.run_bass_kernel_spmd(
        nc, in_maps, core_ids=list(range(NCORES)), trace=_trace
    )
    out = np.empty((B, N, DOUT), dtype=np.float32)
    for c in range(NCORES):
        b, h = divmod(c, 2)
        out[b, h * TPC : (h + 1) * TPC, :] = res.results[c]["y"]
    if _trace:
        kernel.last_exec_time_ns = res.exec_time_ns
        kernel.last_results = res
    return out
